# revision 5
# baseline (speedup 1.0000x reference)
"""CapsuleLayer (dynamic routing) Trainium2 kernel.

Full inputs:  x (32, 2048, 32) f32, W (2048, 64, 32, 32) f32  [W indexed n,j,d,k]
Output:       v (32, 64, 32) f32

Math (reference):
    u_hat[b,j,n,k] = sum_d W[n,j,d,k] * x[b,n,d]
    b = 0; 3 routing iters:
        c = softmax_j(b); s[b,j,k] = sum_n c[b,j,n]*u_hat[b,j,n,k]; v = squash(s)
        b += sum_k u_hat[b,j,n,k]*v[b,j,k]   (first 2 iters)

Sharding: input-capsule axis n split over 8 cores (256 each). Per-core:
  - W slice packed on host to fp16 tiles [g, (n4 d), (j k)]  (g = group of 4 n)
  - x slice packed on host to fp16 [(n4 d), (g b)]
  - u_hat computed once on PE (32x32 tile_position diagonal matmuls), stored
    fp16 in SBUF (resident groups) / DRAM (spilled groups)
  - s accumulated per group on PE via a block-diagonal ones matmul
  - per-iteration 256KB AllReduce of s over the 8 cores; squash computed
    redundantly on every core; v replicated into the (n4 b) partition layout
"""

import os
from contextlib import ExitStack

import numpy as np

B, NTOT, DD, J, K = 32, 2048, 32, 64, 32
JK = J * K
CORES = 8
NL = NTOT // CORES          # input capsules per core
ITERS = 3

_CACHED = {}


def _build_nc(NL_, G_RES, n_cores):
    import concourse.bass as bass
    import concourse.mybir as mybir
    import concourse.tile as tile
    from concourse import bacc
    from concourse.masks import make_identity

    G = NL_ // 4            # groups of 4 input capsules
    G_RES = min(G_RES, G)
    NSPILL = G - G_RES
    f16 = mybir.dt.float16
    f32 = mybir.dt.float32
    AX = mybir.AxisListType
    OP = mybir.AluOpType
    AF = mybir.ActivationFunctionType

    nc = bacc.Bacc()
    wd = nc.declare_dram_parameter("w", [G, 128, JK], f16, isOutput=False)
    xtd = nc.declare_dram_parameter("xt", [128, G * B], f16, isOutput=False)
    vd = nc.declare_dram_parameter("v", [B, JK], f32, isOutput=True)

    core_ids = list(range(n_cores))

    with tile.TileContext(nc) as tc, ExitStack() as ctx:
        const = ctx.enter_context(tc.tile_pool(name="const", bufs=1))
        dram = ctx.enter_context(tc.tile_pool(name="dram", bufs=1, space="DRAM"))
        ures = ctx.enter_context(tc.tile_pool(name="ures", bufs=1))
        sm = ctx.enter_context(tc.tile_pool(name="small", bufs=1))
        smg = ctx.enter_context(tc.tile_pool(name="smallg", bufs=3))
        sv = ctx.enter_context(tc.tile_pool(name="sv", bufs=2))
        vrp = ctx.enter_context(tc.tile_pool(name="vrp", bufs=2))

        # ---- constants ----
        xts = const.tile([128, G * B], f16)
        nc.sync.dma_start(out=xts, in_=xtd[:])
        ident = const.tile([32, 32], f16)
        make_identity(nc, ident)
        odiag = const.tile([128, B], f16)   # odiag[p, b] = 1 if p % 32 == b
        for r in range(4):
            nc.vector.tensor_copy(odiag[32 * r:32 * r + 32, :], ident)

        b_sb = const.tile([128, G * J], f32)        # routing logits per (n4 b)

        if NSPILL:
            u_spill = dram.tile([NSPILL, 128, JK], f16)
        cc_in = dram.tile([B, JK], f32)
        cc_out = dram.tile([B, JK], f32)

        u_tiles = {}

        def u_tile(g):
            if g < G_RES:
                if g not in u_tiles:
                    u_tiles[g] = ures.tile(
                        [128, JK], f16, tag=f"u{g}", name=f"u{g}"
                    )
                return u_tiles[g], True
            return None, False

        # ---------- squash + AllReduce of s; returns v_rep fp16 [128, JK] ----------
        def finish_iteration(s_psum, scale_mul, last):
            s_sb = sm.tile([B, JK], f32, tag="s_work")
            nc.scalar.mul(s_sb, s_psum, scale_mul)
            nc.sync.dma_start(out=cc_in[:], in_=s_sb)
            nc.gpsimd.collective_compute(
                "AllReduce",
                OP.add,
                ins=[cc_in[:].opt()],
                outs=[cc_out[:].opt()],
                replica_groups=[core_ids],
            )
            s_tot = sm.tile([B, JK], f32, tag="s_work", name="s_tot")
            nc.sync.dma_start(out=s_tot, in_=cc_out[:])

            sq = sm.tile([B, JK], f32, tag="tmp1")
            nc.vector.tensor_mul(sq, s_tot, s_tot)
            s2 = sm.tile([B, J], f32, tag="s2")
            nc.vector.tensor_reduce(
                s2, sq.rearrange("b (j k) -> b j k", j=J), axis=AX.X, op=OP.add
            )
            p1 = sm.tile([B, J], f32, tag="p1")
            nc.vector.tensor_scalar_add(p1, s2, 1.0)
            r1 = sm.tile([B, J], f32, tag="r1")
            nc.vector.reciprocal(r1, p1)
            pe_ = sm.tile([B, J], f32, tag="pe")
            nc.vector.tensor_scalar_add(pe_, s2, 1e-8)
            rt = sm.tile([B, J], f32, tag="rt")
            nc.scalar.sqrt(rt, pe_)
            r2 = sm.tile([B, J], f32, tag="r2")
            nc.vector.reciprocal(r2, rt)
            sc = sm.tile([B, J], f32, tag="sc")
            nc.vector.tensor_mul(sc, s2, r1)
            nc.vector.tensor_mul(sc, sc, r2)
            # v = s_tot * sc (sc broadcast over k)
            sc_rep = sm.tile([B, J, K], f32, tag="tmp2")
            sc_b = bass.AP(
                tensor=sc.tensor, offset=sc.offset,
                ap=[sc.ap[0], sc.ap[1], [0, K]],
            )
            nc.scalar.copy(sc_rep, sc_b)
            v_sb = sm.tile([B, JK], f32, tag="v_sb")
            nc.vector.tensor_mul(
                v_sb, s_tot, sc_rep.rearrange("b j k -> b (j k)")
            )
            if last:
                nc.sync.dma_start(out=vd[:], in_=v_sb)
                return None
            v_rep = vrp.tile([128, JK], f16, tag="v_rep")
            for r in range(4):
                eng = nc.vector if r % 2 == 0 else nc.scalar
                if r % 2 == 0:
                    eng.tensor_copy(v_rep[32 * r:32 * r + 32, :], v_sb)
                else:
                    eng.copy(v_rep[32 * r:32 * r + 32, :], v_sb)
            return v_rep

        # ================= pass 1: u_hat + s1 =================
        with tc.tile_pool(name="wp", bufs=3) as wp, \
             tc.tile_pool(name="pu", bufs=2, space="PSUM") as pu, \
             tc.tile_pool(name="ps1", bufs=1, space="PSUM") as ps1, \
             tc.tile_pool(name="ustg1", bufs=2) as ustg1:
            s1_psum = ps1.tile([B, JK], f32)
            for g in range(G):
                wt = wp.tile([128, JK], f16, tag="wt")
                nc.sync.dma_start(out=wt, in_=wd[g])
                ut, resident = u_tile(g)
                if not resident:
                    ut = ustg1.tile([128, JK], f16, tag="ustg")
                xsl = xts[:, g * B:(g + 1) * B]
                for h in range(2):
                    up = pu.tile([128, 1024], f32, tag="up")
                    for cch in range(2):
                        lo = h * 1024 + cch * 512
                        sl = slice(lo, lo + 512)
                        psl = slice(cch * 512, cch * 512 + 512)
                        for r in range(4):
                            rs = slice(32 * r, 32 * r + 32)
                            nc.tensor.matmul(
                                up[rs, psl],
                                lhsT=xsl[rs, :],
                                rhs=wt[rs, sl],
                                start=True, stop=True,
                                tile_position=(32 * r, 32 * r),
                                skip_group_check=True,
                            )
                        nc.tensor.matmul(
                            s1_psum[:, sl],
                            lhsT=xsl,
                            rhs=wt[:, sl],
                            start=(g == 0), stop=(g == G - 1),
                            skip_group_check=True,
                        )
                    osl = slice(h * 1024, h * 1024 + 1024)
                    if h == 0:
                        nc.vector.tensor_copy(ut[:, osl], up)
                    else:
                        nc.scalar.copy(ut[:, osl], up)
                if not resident:
                    nc.sync.dma_start(out=u_spill[g - G_RES], in_=ut)
            v_rep = finish_iteration(s1_psum, 1.0 / J, last=False)

        # ================= passes 2..ITERS =================
        with tc.tile_pool(name="ps23", bufs=1, space="PSUM") as ps23, \
             tc.tile_pool(name="ustg2", bufs=2) as ustg2, \
             tc.tile_pool(name="wtp", bufs=2) as wtp, \
             tc.tile_pool(name="crp", bufs=2) as crp, \
             tc.tile_pool(name="cup", bufs=2) as cup:
            for it in range(1, ITERS):
                s_psum = ps23.tile([B, JK], f32, tag="s23")
                for g in range(G):
                    ut, resident = u_tile(g)
                    if not resident:
                        ut = ustg2.tile([128, JK], f16, tag="ustg2")
                        nc.sync.dma_start(out=ut, in_=u_spill[g - G_RES])
                    w_t = wtp.tile([128, JK], f16, tag="w_t")
                    nc.gpsimd.tensor_mul(w_t, ut, v_rep)
                    bsl = b_sb[:, g * J:(g + 1) * J]
                    if it == 1:
                        nc.vector.tensor_reduce(
                            bsl, w_t.rearrange("p (j k) -> p j k", j=J),
                            axis=AX.X, op=OP.add,
                        )
                    else:
                        t_t = smg.tile([128, J], f32, tag="t_t")
                        nc.vector.tensor_reduce(
                            t_t, w_t.rearrange("p (j k) -> p j k", j=J),
                            axis=AX.X, op=OP.add,
                        )
                        nc.vector.tensor_add(bsl, bsl, t_t)
                    nmx = smg.tile([128, 1], f32, tag="nmx")
                    nc.vector.tensor_reduce(
                        nmx, bsl, axis=AX.X, op=OP.max, negate=True
                    )
                    e_t = smg.tile([128, J], f32, tag="e_t")
                    nc.scalar.activation(e_t, bsl, AF.Exp, bias=nmx, scale=1.0)
                    se = smg.tile([128, 1], f32, tag="se")
                    nc.vector.tensor_reduce(se, e_t, axis=AX.X, op=OP.add)
                    rc = smg.tile([128, 1], f32, tag="rc")
                    nc.vector.reciprocal(rc, se)
                    c_t = smg.tile([128, J], f16, tag="c_t")
                    nc.vector.tensor_scalar_mul(c_t, e_t, rc)
                    c_rep = crp.tile([128, J, K], f16, tag="c_rep")
                    c_b = bass.AP(
                        tensor=c_t.tensor, offset=c_t.offset,
                        ap=[c_t.ap[0], c_t.ap[1], [0, K]],
                    )
                    nc.scalar.copy(c_rep, c_b)
                    cu = cup.tile([128, JK], f16, tag="cu")
                    nc.vector.tensor_mul(
                        cu, c_rep.rearrange("p j k -> p (j k)"), ut
                    )
                    for cch in range(4):
                        sl = slice(cch * 512, cch * 512 + 512)
                        nc.tensor.matmul(
                            s_psum[:, sl],
                            lhsT=odiag,
                            rhs=cu[:, sl],
                            start=(g == 0), stop=(g == G - 1),
                            skip_group_check=True,
                        )
                v_rep = finish_iteration(
                    s_psum, 1.0, last=(it == ITERS - 1)
                )

    nc.finalize()
    return nc


def _pack_inputs(x, W, n_cores):
    """Shard over n, cast fp16, pre-transpose to the on-chip layouts."""
    nl = NTOT // n_cores
    g = nl // 4
    in_maps = []
    for c in range(n_cores):
        wl = W[c * nl:(c + 1) * nl]                       # (nl, J, D, K)
        wp = np.ascontiguousarray(
            wl.reshape(g, 4, J, DD, K).transpose(0, 1, 3, 2, 4)
            .reshape(g, 128, JK).astype(np.float16)
        )
        xl = x[:, c * nl:(c + 1) * nl, :]                 # (B, nl, D)
        xt = np.ascontiguousarray(
            xl.transpose(1, 2, 0).reshape(g, 4, DD, B)    # (g, n4, d, b)
            .reshape(g, 128, B).transpose(1, 0, 2)        # (128, g, b)
            .reshape(128, g * B).astype(np.float16)
        )
        in_maps.append({"w": wp, "xt": xt})
    return in_maps


def kernel(x, W):
    from concourse.bass_utils import run_bass_kernel_spmd

    x = np.asarray(x, dtype=np.float32)
    W = np.asarray(W, dtype=np.float32)
    g_res = int(os.environ.get("CAPS_G_RES", "16"))
    key = (NL, g_res, CORES)
    if key not in _CACHED:
        _CACHED[key] = _build_nc(NL, g_res, CORES)
    nc = _CACHED[key]
    in_maps = _pack_inputs(x, W, CORES)
    res = run_bass_kernel_spmd(nc, in_maps, list(range(CORES)))
    v = np.asarray(res.results[0]["v"], dtype=np.float32)
    return v.reshape(B, J, K)
